# revision 14
# baseline (speedup 1.0000x reference)
# DCN CrossLayer kernel for Trainium2 (8 NeuronCores, data-parallel over batch).
#
# Reference computation (per example row x of length D, L=3 layers):
#   cross = x
#   for i in range(L):
#       s_i   = <cross, W_i>                  (scalar per example)
#       cross = x * s_i + bias_i + cross
#
# Algebraic collapse: cross_i = a_i * x + B_i with per-example scalar a_i and
# batch-independent vector B_i = sum_{j<i} bias_j.  Then
#   s_i     = a_i * t_i + c_i,   t_i = <x, W_i>,  c_i = <B_i, W_i>
#   a_{i+1} = a_i * (1 + t_i) + c_i
#   out     = a_L * x + B_L
# so the device kernel only needs the three dot products t_i = <x, W_i>
# (one skinny matmul against W^T), a tiny per-row recurrence, and one
# per-row scale of x.  c_i and B_L are computed on the host (they do not
# depend on the batch).
#
# The kernel is HBM-bandwidth-bound (~400 GB/s aggregate per core across the
# 16 SDMA engines).  To halve the traffic the device I/O is fp16: the host
# casts x -> f16 before upload and upcasts y f16 -> f32 after download.  The
# dot products already ran in f16 on the PE (error ~5e-4, gate is 2e-2).
#
# Device plan per core (2048 rows of 1024, f16):
#   - rows mapped p-major (row = p*TILES + t) so each partition's DMA run is
#     su contiguous rows (4 KiB at su=2) instead of one row (2 KiB)
#   - DMA x in supertiles [128 part, 2, 1024] f16 on the SP HWDGE ring
#   - PE transposes each [128,128] block of x -> PSUM
#   - ACT copies the transposed tile PSUM -> SBUF as uint32 (f16 pairs, so
#     512 streamed columns instead of 1024)
#   - PE matmuls xt_k^T @ Wt_k accumulating t [128 rows, 3] in PSUM
#   - DVE: a3 = ((1+t0)(1+t1)+c1)(1+t2)+c2 in f16; y = x * a3 (+ B_L), f16
#   - DMA y out on the ACT HWDGE ring (issued one supertile late so the
#     Scalar queue never stalls waiting on fresh DVE output)
import os
from contextlib import ExitStack

import numpy as np

import concourse.bacc as bacc
import concourse.bass as bass
import concourse.tile as tile
from concourse import mybir
from concourse.bass_utils import run_bass_kernel_spmd
from concourse.masks import make_identity

B, D, L = 16384, 1024, 3
N_CORES = 8
ROWS = B // N_CORES  # rows per core
P = 128
TILES = ROWS // P  # 16 row-tiles per core
# supertile schedule: small supers at the ends so the pipeline fills and
# drains quickly; 512 KiB DMAs in the middle
SCHED = [1, 1, 2, 2, 2, 2, 2, 2, 1, 1]
assert sum(SCHED) == TILES
KCH = D // P  # 8 d-chunks of 128

F32 = mybir.dt.float32
F16 = mybir.dt.float16
U32 = mybir.dt.uint32

# test.py can flip these before calling kernel() to get an NTFF profile.
TRACE = False
LAST_RESULT = None


def _build(has_bias: bool, c1: float, c2: float) -> bass.Bass:
    nc = bacc.Bacc("TRN2", target_bir_lowering=False)
    x = nc.dram_tensor("x", [ROWS, D], F16, kind="ExternalInput")
    wt = nc.dram_tensor("wt", [P, KCH, L], F16, kind="ExternalInput")
    if has_bias:
        b3 = nc.dram_tensor("b3", [1, D], F16, kind="ExternalInput")
    y = nc.dram_tensor("y", [ROWS, D], F16, kind="ExternalOutput")

    # row r = p*TILES + t  ->  [p][t][d]; consecutive t are consecutive DRAM
    # rows, so a [:, t0:t0+su, :] DMA moves su*2KiB contiguous per partition
    xv = x.rearrange("(p t) d -> p t d", t=TILES)
    yv = y.rearrange("(p t) d -> p t d", t=TILES)

    with tile.TileContext(nc) as tc, ExitStack() as ctx:
        singles = ctx.enter_context(tc.tile_pool(name="singles", bufs=1))
        xpool = ctx.enter_context(tc.tile_pool(name="xpool", bufs=10))
        opool = ctx.enter_context(tc.tile_pool(name="opool", bufs=4))
        xtpool = ctx.enter_context(tc.tile_pool(name="xtpool", bufs=4))
        small = ctx.enter_context(tc.tile_pool(name="small", bufs=4))
        psA = ctx.enter_context(tc.tile_pool(name="psA", bufs=3, space="PSUM"))
        psB = ctx.enter_context(tc.tile_pool(name="psB", bufs=3, space="PSUM"))

        # tiny constant DMA goes on the SWDGE ring so it cannot delay the
        # first big x in-DMA on the SP HWDGE ring
        wt_sb = singles.tile([P, KCH, L], F16)
        nc.gpsimd.dma_start(out=wt_sb, in_=wt[:])
        eye_sb = singles.tile([P, P], F16)
        make_identity(nc, eye_sb)
        # ones vectors for the "+1" matmul: a rank-1 PE update adds 1.0 to
        # every t entry while it is still accumulating in PSUM
        ones1 = singles.tile([1, P], F16)
        nc.vector.memset(ones1, 1.0)
        ones3 = singles.tile([1, L], F16)
        nc.vector.memset(ones3, 1.0)
        if has_bias:
            b3_sb = singles.tile([P, D], F16)
            b3_bcast = bass.AP(
                tensor=b3.tensor, offset=b3.offset, ap=[[0, P], b3.ap[1]]
            )
            nc.gpsimd.dma_start(out=b3_sb, in_=b3_bcast)

        pending = None  # (ys tile, t_off, su) awaiting out-DMA
        t_off = 0
        tidx = 0  # global row-tile counter (for engine alternation)
        for s, su in enumerate(SCHED):
            last = s == len(SCHED) - 1
            xs = xpool.tile([P, 2, D], F16, tag="xs")
            nc.sync.dma_start(out=xs[:, :su, :], in_=xv[:, t_off : t_off + su, :])
            ys = opool.tile([P, 2, D], F16, tag="ys")
            for u in range(su):
                # transpose x tile: 8 x [128,128] blocks -> psum
                pxt = psA.tile([P, KCH, P], F16)
                for k in range(KCH):
                    nc.tensor.transpose(
                        pxt[:, k, :], xs[:, u, k * P : (k + 1) * P], eye_sb
                    )
                # PSUM -> SBUF move of the transposed tile; alternate the
                # engine per tile so neither ACT nor DVE becomes the pacer.
                # The DVE copy streams uint32 (f16 pairs -> 512 columns).
                xt = xtpool.tile([P, KCH, P], F16)
                if tidx % 4 == 3:
                    nc.vector.tensor_copy(xt.bitcast(U32), pxt.bitcast(U32))
                else:
                    nc.scalar.copy(out=xt, in_=pxt)
                if u == 0 and pending is not None:
                    # previous supertile's result is long since computed;
                    # issuing here keeps the Scalar queue stall-free
                    pys, p_off, p_su = pending
                    nc.scalar.dma_start(
                        out=yv[:, p_off : p_off + p_su, :], in_=pys[:, :p_su, :]
                    )
                    pending = None
                # t[row, l] = 1 + sum_d x[row, d] * W[l, d]: 8 accumulating
                # chunk matmuls plus a rank-1 ones update for the +1
                pt = psB.tile([P, L], F32)
                for k in range(KCH):
                    nc.tensor.matmul(
                        pt,
                        xt[:, k, :],
                        wt_sb[:, k, :],
                        start=(k == 0),
                        stop=False,
                    )
                nc.tensor.matmul(pt, ones1, ones3, start=False, stop=True)
                # a3 = (u0*u1+c1)*u2+c2; DVE TT can read at most one PSUM
                # operand, so bounce u0/u1 through SBUF first (tiny copy)
                ut = small.tile([P, 2], F32, tag="ut")
                nc.vector.tensor_copy(ut, pt[:, 0:2])
                m01 = small.tile([P, 1], F32, tag="m01")
                nc.vector.tensor_mul(m01, ut[:, 0:1], ut[:, 1:2])
                if c1 != 0.0:
                    nc.vector.tensor_scalar_add(m01, m01, c1)
                a3 = small.tile([P, 1], F32, tag="a3")
                nc.vector.tensor_mul(a3, m01, pt[:, 2:3])
                if c2 != 0.0:
                    nc.vector.tensor_scalar_add(a3, a3, c2)
                # out = x * a3 (+ B_L); all-f16 single-src op on DVE
                nc.vector.tensor_scalar_mul(ys[:, u, :], xs[:, u, :], a3)
                if has_bias:
                    nc.vector.tensor_add(ys[:, u, :], ys[:, u, :], b3_sb)
                tidx += 1
            if last:
                if pending is not None:
                    pys, p_off, p_su = pending
                    nc.scalar.dma_start(
                        out=yv[:, p_off : p_off + p_su, :], in_=pys[:, :p_su, :]
                    )
                nc.scalar.dma_start(
                    out=yv[:, t_off : t_off + su, :], in_=ys[:, :su, :]
                )
                pending = None
            else:
                pending = (ys, t_off, su)
            t_off += su
    nc.finalize()
    return nc


def kernel(x, W, bias):
    global LAST_RESULT
    x2 = np.asarray(x, dtype=np.float32).reshape(B, D)
    W2 = np.asarray(W, dtype=np.float32).reshape(L, D)
    B2 = np.asarray(bias, dtype=np.float32).reshape(L, D)

    # host-side constants
    has_bias = bool(np.any(B2 != 0.0))
    c1 = float(B2[0] @ W2[1])
    c2 = float((B2[0] + B2[1]) @ W2[2])
    b3_host = np.ascontiguousarray(B2.sum(axis=0).reshape(1, D).astype(np.float16))
    # wt[p, k, l] = W[l, k*128 + p]
    wt_host = np.ascontiguousarray(
        W2.T.reshape(KCH, P, L).transpose(1, 0, 2).astype(np.float16)
    )

    nc = _build(has_bias, c1 if has_bias else 0.0, c2 if has_bias else 0.0)

    x16 = np.ascontiguousarray(x2.astype(np.float16))
    shards = np.split(x16, N_CORES, axis=0)
    in_maps = []
    for c in range(N_CORES):
        m = {"x": shards[c], "wt": wt_host}
        if has_bias:
            m["b3"] = b3_host
        in_maps.append(m)

    kwargs = {}
    if TRACE:
        kwargs = dict(trace=True, trace_cores=[0])
    res = run_bass_kernel_spmd(nc, in_maps, core_ids=list(range(N_CORES)), **kwargs)
    LAST_RESULT = res
    out = np.concatenate(
        [res.results[c]["y"].astype(np.float32) for c in range(N_CORES)], axis=0
    )
    return np.ascontiguousarray(out.reshape(B, D, 1))


# revision 17
# speedup vs baseline: 1.0866x; 1.0866x over previous
# DCN CrossLayer kernel for Trainium2 (8 NeuronCores, data-parallel over batch).
#
# Reference computation (per example row x of length D, L=3 layers):
#   cross = x
#   for i in range(L):
#       s_i   = <cross, W_i>                  (scalar per example)
#       cross = x * s_i + bias_i + cross
#
# Algebraic collapse: cross_i = a_i * x + B_i with per-example scalar a_i and
# batch-independent vector B_i = sum_{j<i} bias_j.  Then
#   s_i     = a_i * t_i + c_i,   t_i = <x, W_i>,  c_i = <B_i, W_i>
#   a_{i+1} = a_i * (1 + t_i) + c_i
#   out     = a_L * x + B_L
# so the device kernel only needs the three dot products t_i = <x, W_i>
# (one skinny matmul against W^T), a tiny per-row recurrence, and one
# per-row scale of x.  c_i and B_L are computed on the host (they do not
# depend on the batch).
#
# The kernel is HBM-bandwidth-bound (~400 GB/s aggregate per core across the
# 16 SDMA engines).  To halve the traffic the device I/O is fp16: the host
# casts x -> f16 before upload and upcasts y f16 -> f32 after download.  The
# dot products already ran in f16 on the PE (error ~5e-4, gate is 2e-2).
#
# Device plan per core (2048 rows of 1024, f16):
#   - rows mapped p-major (row = p*TILES + t) so each partition's DMA run is
#     su contiguous rows (4 KiB at su=2) instead of one row (2 KiB)
#   - DMA x in supertiles [128 part, 2, 1024] f16 on the SP HWDGE ring
#   - PE transposes each [128,128] block of x -> PSUM
#   - ACT copies the transposed tile PSUM -> SBUF as uint32 (f16 pairs, so
#     512 streamed columns instead of 1024)
#   - PE matmuls xt_k^T @ Wt_k accumulating t [128 rows, 3] in PSUM
#   - DVE: a3 = ((1+t0)(1+t1)+c1)(1+t2)+c2 in f16; y = x * a3 (+ B_L), f16
#   - DMA y out on the ACT HWDGE ring (issued one supertile late so the
#     Scalar queue never stalls waiting on fresh DVE output)
import os
from contextlib import ExitStack

import numpy as np

import concourse.bacc as bacc
import concourse.bass as bass
import concourse.tile as tile
from concourse import mybir
from concourse.bass_utils import run_bass_kernel_spmd
from concourse.masks import make_identity

B, D, L = 16384, 1024, 3
N_CORES = 8
ROWS = B // N_CORES  # rows per core
P = 128
TILES = ROWS // P  # 16 row-tiles per core
# supertile schedule: small supers at the ends so the pipeline fills and
# drains quickly; big supers in the middle keep the per-engine queues deep
SCHED = [1, 1, 2, 4, 4, 2, 1, 1]
assert sum(SCHED) == TILES
KCH = D // P  # 8 d-chunks of 128

F32 = mybir.dt.float32
F16 = mybir.dt.float16
U32 = mybir.dt.uint32

# test.py can flip these before calling kernel() to get an NTFF profile.
TRACE = False
LAST_RESULT = None


def _build(has_bias: bool, c1: float, c2: float) -> bass.Bass:
    nc = bacc.Bacc("TRN2", target_bir_lowering=False)
    x = nc.dram_tensor("x", [ROWS, D], F16, kind="ExternalInput")
    wt = nc.dram_tensor("wt", [P, KCH, L], F16, kind="ExternalInput")
    if has_bias:
        b3 = nc.dram_tensor("b3", [1, D], F16, kind="ExternalInput")
    y = nc.dram_tensor("y", [ROWS, D], F16, kind="ExternalOutput")

    # row r = p*TILES + t  ->  [p][t][d]; consecutive t are consecutive DRAM
    # rows, so a [:, t0:t0+su, :] DMA moves su*2KiB contiguous per partition
    xv = x.rearrange("(p t) d -> p t d", t=TILES)
    yv = y.rearrange("(p t) d -> p t d", t=TILES)

    with tile.TileContext(nc) as tc, ExitStack() as ctx:
        singles = ctx.enter_context(tc.tile_pool(name="singles", bufs=1))
        xpool = ctx.enter_context(tc.tile_pool(name="xpool", bufs=8))
        opool = ctx.enter_context(tc.tile_pool(name="opool", bufs=4))
        xtpool = ctx.enter_context(tc.tile_pool(name="xtpool", bufs=4))
        small = ctx.enter_context(tc.tile_pool(name="small", bufs=4))
        psA = ctx.enter_context(tc.tile_pool(name="psA", bufs=3, space="PSUM"))
        psB = ctx.enter_context(tc.tile_pool(name="psB", bufs=3, space="PSUM"))

        # tiny constant DMA goes on the SWDGE ring so it cannot delay the
        # first big x in-DMA on the SP HWDGE ring
        wt_sb = singles.tile([P, KCH, L], F16)
        nc.gpsimd.dma_start(out=wt_sb, in_=wt[:])
        eye_sb = singles.tile([P, P], F16)
        make_identity(nc, eye_sb)
        # ones vectors for the "+1" matmul: a rank-1 PE update adds 1.0 to
        # every t entry while it is still accumulating in PSUM
        ones1 = singles.tile([1, P], F16)
        nc.vector.memset(ones1, 1.0)
        ones3 = singles.tile([1, L], F16)
        nc.vector.memset(ones3, 1.0)
        if has_bias:
            b3_sb = singles.tile([P, D], F16)
            b3_bcast = bass.AP(
                tensor=b3.tensor, offset=b3.offset, ap=[[0, P], b3.ap[1]]
            )
            nc.gpsimd.dma_start(out=b3_sb, in_=b3_bcast)

        # One-tile software pipeline: stage A (transpose/copy/dots) for tile
        # n is emitted before stage B (recurrence + scale) for tile n-1, so
        # each engine queue sees work whose inputs are already a tile old.
        pending = []  # (ys tile, t_off, su, ready_tidx) awaiting out-DMA
        prev = None  # stage-B state for tile n-1: (pt, xs, ys, u)

        def stage_b(st):
            pt, bxs, bys, bu = st
            # a3 = (u0*u1+c1)*u2+c2; DVE TT can read at most one PSUM
            # operand, so ACT bounces u0/u1 through SBUF (tiny copy)
            ut = small.tile([P, 2], F32, tag="ut")
            nc.scalar.copy(out=ut, in_=pt[:, 0:2])
            m01 = small.tile([P, 1], F32, tag="m01")
            nc.vector.tensor_mul(m01, ut[:, 0:1], ut[:, 1:2])
            if c1 != 0.0:
                nc.vector.tensor_scalar_add(m01, m01, c1)
            a3 = small.tile([P, 1], F32, tag="a3")
            nc.vector.tensor_mul(a3, m01, pt[:, 2:3])
            if c2 != 0.0:
                nc.vector.tensor_scalar_add(a3, a3, c2)
            # out = x * a3 (+ B_L); all-f16 single-src op on DVE
            nc.vector.tensor_scalar_mul(bys[:, bu, :], bxs[:, bu, :], a3)
            if has_bias:
                nc.vector.tensor_add(bys[:, bu, :], bys[:, bu, :], b3_sb)

        t_off = 0
        tidx = 0  # global row-tile counter
        for s, su in enumerate(SCHED):
            xs = xpool.tile([P, 4, D], F16, tag="xs")
            nc.sync.dma_start(out=xs[:, :su, :], in_=xv[:, t_off : t_off + su, :])
            ys = opool.tile([P, 4, D], F16, tag="ys")
            for u in range(su):
                # flush out-DMAs whose data was computed >= 2 tiles ago, so
                # the Scalar queue never stalls on a fresh DVE result
                while pending and tidx >= pending[0][3] + 2:
                    pys, p_off, p_su, _ = pending.pop(0)
                    nc.scalar.dma_start(
                        out=yv[:, p_off : p_off + p_su, :], in_=pys[:, :p_su, :]
                    )
                # transpose x tile: 8 x [128,128] blocks -> psum
                pxt = psA.tile([P, KCH, P], F16)
                for k in range(KCH):
                    nc.tensor.transpose(
                        pxt[:, k, :], xs[:, u, k * P : (k + 1) * P], eye_sb
                    )
                # PSUM -> SBUF move of the transposed tile; one tile in four
                # goes to DVE (as uint32: f16 pairs -> 512 columns) so the
                # ACT engine stays under the DMA-wire pace
                xt = xtpool.tile([P, KCH, P], F16)
                if tidx % 4 == 2:
                    nc.vector.tensor_copy(xt.bitcast(U32), pxt.bitcast(U32))
                else:
                    nc.scalar.copy(out=xt, in_=pxt)
                # t[row, l] = 1 + sum_d x[row, d] * W[l, d]: 8 accumulating
                # chunk matmuls plus a rank-1 ones update for the +1
                pt = psB.tile([P, L], F32)
                for k in range(KCH):
                    nc.tensor.matmul(
                        pt,
                        xt[:, k, :],
                        wt_sb[:, k, :],
                        start=(k == 0),
                        stop=False,
                    )
                nc.tensor.matmul(pt, ones1, ones3, start=False, stop=True)
                if prev is not None:
                    stage_b(prev)
                prev = (pt, xs, ys, u)
                tidx += 1
            pending.append((ys, t_off, su, tidx))
            t_off += su
        stage_b(prev)
        for pys, p_off, p_su, _ in pending:
            nc.scalar.dma_start(
                out=yv[:, p_off : p_off + p_su, :], in_=pys[:, :p_su, :]
            )
    nc.finalize()
    return nc


def kernel(x, W, bias):
    global LAST_RESULT
    x2 = np.asarray(x, dtype=np.float32).reshape(B, D)
    W2 = np.asarray(W, dtype=np.float32).reshape(L, D)
    B2 = np.asarray(bias, dtype=np.float32).reshape(L, D)

    # host-side constants
    has_bias = bool(np.any(B2 != 0.0))
    c1 = float(B2[0] @ W2[1])
    c2 = float((B2[0] + B2[1]) @ W2[2])
    b3_host = np.ascontiguousarray(B2.sum(axis=0).reshape(1, D).astype(np.float16))
    # wt[p, k, l] = W[l, k*128 + p]
    wt_host = np.ascontiguousarray(
        W2.T.reshape(KCH, P, L).transpose(1, 0, 2).astype(np.float16)
    )

    nc = _build(has_bias, c1 if has_bias else 0.0, c2 if has_bias else 0.0)

    x16 = np.ascontiguousarray(x2.astype(np.float16))
    shards = np.split(x16, N_CORES, axis=0)
    in_maps = []
    for c in range(N_CORES):
        m = {"x": shards[c], "wt": wt_host}
        if has_bias:
            m["b3"] = b3_host
        in_maps.append(m)

    kwargs = {}
    if TRACE:
        kwargs = dict(trace=True, trace_cores=[0])
    res = run_bass_kernel_spmd(nc, in_maps, core_ids=list(range(N_CORES)), **kwargs)
    LAST_RESULT = res
    out = np.concatenate(
        [res.results[c]["y"].astype(np.float32) for c in range(N_CORES)], axis=0
    )
    return np.ascontiguousarray(out.reshape(B, D, 1))
